# revision 5
# baseline (speedup 1.0000x reference)
"""Multi-head attention (16 heads, d_model=1024, B=2, S=2048) on 8 Trainium2
NeuronCores, tensor-parallel over heads (2 heads per core).

Per-core program (all matmuls bf16 with fp32 PSUM accumulation):
  - q_T/k_T = (W X^T + b) computed in transposed [d, token] layout
  - v in natural [token, d] layout with a ones-column appended (gives the
    softmax denominators for free from the same attn@v matmul)
  - scores_T[j, q] = k_T^T-stationary matmul, exp on ScalarE straight out of
    PSUM (softmax without max-subtraction: scores ~ N(0,1), no overflow risk)
  - unnormalized attn output + denominators accumulate in PSUM; normalization
    applied during eviction via a partition-broadcast reciprocal
  - row block of Wo produces a partial [B*S, 1024] output per core
Host: sum of the 8 partials + (bv @ Wo^T + bo) correction (exact because
softmax rows sum to 1, so the V-bias commutes out of attention).
"""

import numpy as np
import ml_dtypes

import concourse.bass as bass
import concourse.tile as tile
import concourse.bacc as bacc
from concourse import mybir
from concourse import bass_utils

BF16 = ml_dtypes.bfloat16

D_MODEL = 1024
NUM_HEADS = 16
DK = 64
B, S = 2, 2048
BS = B * S
N_CORES = 8
HPC = NUM_HEADS // N_CORES          # heads per core = 2
DPC = HPC * DK                      # head-dim slice per core = 128
P = 128
NF = D_MODEL // P                   # 8 contraction tiles for projections
NIT = BS // P                       # 32 token tiles of 128
SJT = S // P                        # 16 key tiles per batch
FREE = 1024                         # moving free-dim for bf16 matmuls
NQC = BS // FREE                    # 4 projection column chunks
NQT = S // FREE                     # 2 query chunks per batch

f32 = mybir.dt.float32
bf16 = mybir.dt.bfloat16


def _emit(tc, aps, loop=1):
    nc = tc.nc
    xq, xk, xv, wq, wk, wv, wo, bq, bk, out = aps

    import contextlib
    with contextlib.ExitStack() as ctx:
        const = ctx.enter_context(tc.tile_pool(name="const", bufs=1))
        xpool = ctx.enter_context(tc.tile_pool(name="xpool", bufs=11))
        persist = ctx.enter_context(tc.tile_pool(name="persist", bufs=1))
        exp_pool = ctx.enter_context(tc.tile_pool(name="exp", bufs=3))
        bc_pool = ctx.enter_context(tc.tile_pool(name="bcast", bufs=2))
        rc_pool = ctx.enter_context(tc.tile_pool(name="recip", bufs=2))
        out_pool = ctx.enter_context(tc.tile_pool(name="outp", bufs=3))
        pp_big = ctx.enter_context(tc.tile_pool(name="pp_big", bufs=2, space="PSUM"))
        pp_v = ctx.enter_context(tc.tile_pool(name="pp_v", bufs=2, space="PSUM"))
        pp_av = ctx.enter_context(tc.tile_pool(name="pp_av", bufs=1, space="PSUM"))

        # ---- constants ----
        wq_sb = const.tile([P, NF, P], bf16)
        wk_sb = const.tile([P, NF, P], bf16)
        wv_sb = const.tile([P, NF, P], bf16)
        wo_sb = const.tile([P, D_MODEL], bf16)
        for w_sb, w_ap in ((wq_sb, wq), (wk_sb, wk), (wv_sb, wv)):
            nc.sync.dma_start(w_sb[:], w_ap.rearrange("(n p) m -> p n m", p=P))
        nc.sync.dma_start(wo_sb[:], wo[:])
        bq_sb = const.tile([P, 1], f32)
        bk_sb = const.tile([P, 1], f32)
        nc.sync.dma_start(bq_sb[:], bq[:])
        nc.sync.dma_start(bk_sb[:], bk[:])

        q_sb = persist.tile([P, BS], bf16)
        k_sb = persist.tile([P, BS], bf16)
        v_sb = persist.tile([P, NIT, 2 * (DK + 1)], bf16)
        attn_sb = persist.tile([P, BS], bf16)

        # ones columns of v_aug (softmax denominator rows)
        nc.vector.memset(v_sb[:, :, DK : DK + 1], 1.0)
        nc.vector.memset(v_sb[:, :, 2 * DK + 1 : 2 * DK + 2], 1.0)

        for _ in range(loop):
            # ---- phase 1a: q/k projections (transposed layout) ----
            for w_sb, b_sb, x_ap, dest in (
                (wq_sb, bq_sb, xq, q_sb),
                (wk_sb, bk_sb, xk, k_sb),
            ):
                src = x_ap.rearrange("(n p) m -> n p m", p=P)
                x_tiles = []
                for f in range(NF):
                    t = xpool.tile([P, BS], bf16, tag="x")
                    nc.sync.dma_start(t[:], src[f])
                    x_tiles.append(t)
                for c in range(NQC):
                    ps = pp_big.tile([P, FREE], f32, tag="big")
                    for sub in range(FREE // 512):
                        cs = slice(c * FREE + sub * 512, c * FREE + (sub + 1) * 512)
                        for f in range(NF):
                            nc.tensor.matmul(
                                ps[:, sub * 512 : (sub + 1) * 512],
                                w_sb[:, f, :], x_tiles[f][:, cs],
                                start=(f == 0), stop=(f == NF - 1),
                            )
                    cs = slice(c * FREE, (c + 1) * FREE)
                    nc.vector.tensor_scalar_add(dest[:, cs], ps[:], b_sb[:])

            # ---- phase 1b: v projection (natural layout, + ones cols) ----
            srcv = xv.rearrange("(n p) m -> n p m", p=P)
            xv_tiles = []
            for f in range(NF):
                t = xpool.tile([P, BS], bf16, tag="x")
                nc.sync.dma_start(t[:], srcv[f])
                xv_tiles.append(t)
            for it in range(NIT):
                ps = pp_v.tile([P, P], f32)
                isl = slice(it * P, (it + 1) * P)
                for f in range(NF):
                    nc.tensor.matmul(
                        ps[:], xv_tiles[f][:, isl], wv_sb[:, f, :],
                        start=(f == 0), stop=(f == NF - 1),
                    )
                # write [128, 128] psum into v_sb cols {0:64, 65:129}
                dst = v_sb[:, it, 0:DK]
                dst = bass.AP(dst.tensor, dst.offset, [dst.ap[0], [DK + 1, 2], [1, DK]])
                nc.vector.tensor_copy(dst, ps.rearrange("p (a b) -> p a b", a=2))

            # ---- phase 2: attention per (batch, head) ----
            for b in range(B):
                for h in range(HPC):
                    hsl = slice(h * DK, (h + 1) * DK)
                    for qt in range(NQT):
                        qsl = slice(b * S + qt * FREE, b * S + (qt + 1) * FREE)
                        pav = pp_av.tile([DK + 1, FREE], f32)
                        for jt in range(SJT):
                            jsl = slice(b * S + jt * P, b * S + (jt + 1) * P)
                            pscore = pp_big.tile([P, FREE], f32, tag="big")
                            for sub in range(FREE // 512):
                                ss = slice(sub * 512, (sub + 1) * 512)
                                qss = slice(qsl.start + sub * 512, qsl.start + (sub + 1) * 512)
                                nc.tensor.matmul(
                                    pscore[:, ss], k_sb[hsl, jsl], q_sb[hsl, qss],
                                    start=True, stop=True,
                                )
                            et = exp_pool.tile([P, FREE], bf16)
                            nc.scalar.activation(
                                et[:], pscore[:],
                                mybir.ActivationFunctionType.Exp, scale=0.125,
                            )
                            for sub in range(FREE // 512):
                                ss = slice(sub * 512, (sub + 1) * 512)
                                nc.tensor.matmul(
                                    pav[0 : DK + 1, ss],
                                    v_sb[:, b * SJT + jt, h * (DK + 1) : (h + 1) * (DK + 1)],
                                    et[:, ss],
                                    start=(jt == 0), stop=(jt == SJT - 1),
                                )
                        rc = rc_pool.tile([1, FREE], f32)
                        nc.vector.reciprocal(rc[:], pav[DK : DK + 1, :])
                        bc = bc_pool.tile([DK, FREE], f32)
                        nc.gpsimd.partition_broadcast(bc[:], rc[:])
                        nc.vector.tensor_mul(attn_sb[hsl, qsl], pav[0:DK, :], bc[:])

            # ---- phase 3: output projection (row-sharded Wo -> partial) ----
            for it in range(NIT):
                isl = slice(it * P, (it + 1) * P)
                po = pp_big.tile([P, FREE], f32, tag="big")
                for sub in range(FREE // 512):
                    ss = slice(sub * 512, (sub + 1) * 512)
                    nc.tensor.matmul(po[:, ss], attn_sb[:, isl], wo_sb[:, ss],
                                     start=True, stop=True)
                ot = out_pool.tile([P, D_MODEL], f32)
                nc.vector.tensor_copy(ot[:], po[:])
                nc.sync.dma_start(out[isl, :], ot[:])


def _build(loop=1):
    nc = bacc.Bacc("TRN2", target_bir_lowering=False, debug=False,
                   num_devices=N_CORES)
    xq = nc.dram_tensor("xq_t", [D_MODEL, BS], bf16, kind="ExternalInput").ap()
    xk = nc.dram_tensor("xk_t", [D_MODEL, BS], bf16, kind="ExternalInput").ap()
    xv = nc.dram_tensor("xv_t", [D_MODEL, BS], bf16, kind="ExternalInput").ap()
    wq = nc.dram_tensor("wq_t", [D_MODEL, DPC], bf16, kind="ExternalInput").ap()
    wk = nc.dram_tensor("wk_t", [D_MODEL, DPC], bf16, kind="ExternalInput").ap()
    wv = nc.dram_tensor("wv_t", [D_MODEL, DPC], bf16, kind="ExternalInput").ap()
    wo = nc.dram_tensor("wo_t", [DPC, D_MODEL], bf16, kind="ExternalInput").ap()
    bq = nc.dram_tensor("bq", [DPC, 1], f32, kind="ExternalInput").ap()
    bk = nc.dram_tensor("bk", [DPC, 1], f32, kind="ExternalInput").ap()
    out = nc.dram_tensor("out_p", [BS, D_MODEL], f32, kind="ExternalOutput").ap()

    with tile.TileContext(nc) as tc:
        _emit(tc, (xq, xk, xv, wq, wk, wv, wo, bq, bk, out), loop=loop)
    nc.compile()
    return nc


_cache = {}


def _get_nc(loop=1):
    if loop not in _cache:
        _cache[loop] = _build(loop)
    return _cache[loop]


def _make_in_maps(Q, K, V, Wq, bq, Wk, bk, Wv, bv, Wo, bo):
    xq_t = np.ascontiguousarray(np.asarray(Q, np.float32).reshape(BS, D_MODEL).T).astype(BF16)
    xk_t = np.ascontiguousarray(np.asarray(K, np.float32).reshape(BS, D_MODEL).T).astype(BF16)
    xv_t = np.ascontiguousarray(np.asarray(V, np.float32).reshape(BS, D_MODEL).T).astype(BF16)
    in_maps = []
    for c in range(N_CORES):
        sl = slice(c * DPC, (c + 1) * DPC)
        in_maps.append({
            "xq_t": xq_t, "xk_t": xk_t, "xv_t": xv_t,
            "wq_t": np.ascontiguousarray(np.asarray(Wq)[sl].T).astype(BF16),
            "wk_t": np.ascontiguousarray(np.asarray(Wk)[sl].T).astype(BF16),
            "wv_t": np.ascontiguousarray(np.asarray(Wv)[sl].T).astype(BF16),
            "wo_t": np.ascontiguousarray(np.asarray(Wo)[:, sl].T).astype(BF16),
            "bq": np.asarray(bq, np.float32)[sl].reshape(DPC, 1).copy(),
            "bk": np.asarray(bk, np.float32)[sl].reshape(DPC, 1).copy(),
        })
    return in_maps


def kernel(Q, K, V, Wq, bq, Wk, bk, Wv, bv, Wo, bo):
    nc = _get_nc()
    in_maps = _make_in_maps(Q, K, V, Wq, bq, Wk, bk, Wv, bv, Wo, bo)
    res = bass_utils.run_bass_kernel_spmd(nc, in_maps, core_ids=list(range(N_CORES)))
    acc = np.zeros((BS, D_MODEL), np.float32)
    for c in range(N_CORES):
        acc += res.results[c]["out_p"]
    corr = (np.asarray(bv, np.float64) @ np.asarray(Wo, np.float64).T
            + np.asarray(bo, np.float64)).astype(np.float32)
    return (acc + corr[None, :]).reshape(B, S, D_MODEL).astype(np.float32)


# revision 6
# speedup vs baseline: 656.8518x; 656.8518x over previous
"""Multi-head attention (16 heads, d_model=1024, B=2, S=2048) on 8 Trainium2
NeuronCores, tensor-parallel over heads (2 heads per core).

Per-core program (all matmuls bf16 with fp32 PSUM accumulation):
  - q_T/k_T = (W X^T + b) computed in transposed [d, token] layout
  - v in natural [token, d] layout with a ones-column appended (gives the
    softmax denominators for free from the same attn@v matmul)
  - scores_T[j, q] = k_T^T-stationary matmul, exp on ScalarE straight out of
    PSUM (softmax without max-subtraction: scores ~ N(0,1), no overflow risk)
  - unnormalized attn output + denominators accumulate in PSUM; normalization
    applied during eviction via a partition-broadcast reciprocal
  - row block of Wo produces a partial [B*S, 1024] output per core
Host: sum of the 8 partials + (bv @ Wo^T + bo) correction (exact because
softmax rows sum to 1, so the V-bias commutes out of attention).
"""

import numpy as np
import ml_dtypes

import concourse.bass as bass
import concourse.tile as tile
import concourse.bacc as bacc
from concourse import mybir
from concourse import bass_utils

BF16 = ml_dtypes.bfloat16

D_MODEL = 1024
NUM_HEADS = 16
DK = 64
B, S = 2, 2048
BS = B * S
N_CORES = 8
HPC = NUM_HEADS // N_CORES          # heads per core = 2
DPC = HPC * DK                      # head-dim slice per core = 128
P = 128
NF = D_MODEL // P                   # 8 contraction tiles for projections
NIT = BS // P                       # 32 token tiles of 128
SJT = S // P                        # 16 key tiles per batch
FREE = 1024                         # moving free-dim for bf16 matmuls
NQC = BS // FREE                    # 4 projection column chunks
NQT = S // FREE                     # 2 query chunks per batch

f32 = mybir.dt.float32
bf16 = mybir.dt.bfloat16


def _emit(tc, aps, loop=1):
    nc = tc.nc
    xq, xk, xv, wq, wk, wv, wo, bq, bk, out = aps

    import contextlib
    with contextlib.ExitStack() as ctx:
        const = ctx.enter_context(tc.tile_pool(name="const", bufs=1))
        xpool = ctx.enter_context(tc.tile_pool(name="xpool", bufs=11))
        persist = ctx.enter_context(tc.tile_pool(name="persist", bufs=1))
        exp_pool = ctx.enter_context(tc.tile_pool(name="exp", bufs=3))
        bc_pool = ctx.enter_context(tc.tile_pool(name="bcast", bufs=2))
        rc_pool = ctx.enter_context(tc.tile_pool(name="recip", bufs=2))
        out_pool = ctx.enter_context(tc.tile_pool(name="outp", bufs=3))
        pp_big = ctx.enter_context(tc.tile_pool(name="pp_big", bufs=2, space="PSUM"))
        pp_v = ctx.enter_context(tc.tile_pool(name="pp_v", bufs=2, space="PSUM"))
        pp_av = ctx.enter_context(tc.tile_pool(name="pp_av", bufs=1, space="PSUM"))

        # ---- constants ----
        wq_sb = const.tile([P, NF, P], bf16)
        wk_sb = const.tile([P, NF, P], bf16)
        wv_sb = const.tile([P, NF, P], bf16)
        wo_sb = const.tile([P, D_MODEL], bf16)
        for w_sb, w_ap in ((wq_sb, wq), (wk_sb, wk), (wv_sb, wv)):
            nc.sync.dma_start(w_sb[:], w_ap.rearrange("(n p) m -> p n m", p=P))
        nc.sync.dma_start(wo_sb[:], wo[:])
        bq_sb = const.tile([P, 1], f32)
        bk_sb = const.tile([P, 1], f32)
        nc.sync.dma_start(bq_sb[:], bq[:])
        nc.sync.dma_start(bk_sb[:], bk[:])

        q_sb = persist.tile([P, BS], bf16)
        k_sb = persist.tile([P, BS], bf16)
        v_sb = persist.tile([P, NIT, 2 * (DK + 1)], bf16)
        attn_sb = persist.tile([P, BS], bf16)

        # ones columns of v_aug (softmax denominator rows)
        nc.vector.memset(v_sb[:, :, DK : DK + 1], 1.0)
        nc.vector.memset(v_sb[:, :, 2 * DK + 1 : 2 * DK + 2], 1.0)

        import contextlib as _ctl
        loop_cm = tc.For_i(0, loop, 1) if loop > 1 else _ctl.nullcontext()
        with loop_cm:
            # ---- phase 1a: q/k projections (transposed layout) ----
            for w_sb, b_sb, x_ap, dest in (
                (wq_sb, bq_sb, xq, q_sb),
                (wk_sb, bk_sb, xk, k_sb),
            ):
                src = x_ap.rearrange("(n p) m -> n p m", p=P)
                x_tiles = []
                for f in range(NF):
                    t = xpool.tile([P, BS], bf16, tag="x")
                    nc.sync.dma_start(t[:], src[f])
                    x_tiles.append(t)
                for c in range(NQC):
                    ps = pp_big.tile([P, FREE], f32, tag="big")
                    for sub in range(FREE // 512):
                        cs = slice(c * FREE + sub * 512, c * FREE + (sub + 1) * 512)
                        for f in range(NF):
                            nc.tensor.matmul(
                                ps[:, sub * 512 : (sub + 1) * 512],
                                w_sb[:, f, :], x_tiles[f][:, cs],
                                start=(f == 0), stop=(f == NF - 1),
                            )
                    cs = slice(c * FREE, (c + 1) * FREE)
                    nc.vector.tensor_scalar_add(dest[:, cs], ps[:], b_sb[:])

            # ---- phase 1b: v projection (natural layout, + ones cols) ----
            srcv = xv.rearrange("(n p) m -> n p m", p=P)
            xv_tiles = []
            for f in range(NF):
                t = xpool.tile([P, BS], bf16, tag="x")
                nc.sync.dma_start(t[:], srcv[f])
                xv_tiles.append(t)
            for it in range(NIT):
                ps = pp_v.tile([P, P], f32)
                isl = slice(it * P, (it + 1) * P)
                for f in range(NF):
                    nc.tensor.matmul(
                        ps[:], xv_tiles[f][:, isl], wv_sb[:, f, :],
                        start=(f == 0), stop=(f == NF - 1),
                    )
                # write [128, 128] psum into v_sb cols {0:64, 65:129}
                dst = v_sb[:, it, 0:DK]
                dst = bass.AP(dst.tensor, dst.offset, [dst.ap[0], [DK + 1, 2], [1, DK]])
                nc.vector.tensor_copy(dst, ps.rearrange("p (a b) -> p a b", a=2))

            # ---- phase 2: attention per (batch, head) ----
            for b in range(B):
                for h in range(HPC):
                    hsl = slice(h * DK, (h + 1) * DK)
                    for qt in range(NQT):
                        qsl = slice(b * S + qt * FREE, b * S + (qt + 1) * FREE)
                        pav = pp_av.tile([DK + 1, FREE], f32)
                        for jt in range(SJT):
                            jsl = slice(b * S + jt * P, b * S + (jt + 1) * P)
                            pscore = pp_big.tile([P, FREE], f32, tag="big")
                            for sub in range(FREE // 512):
                                ss = slice(sub * 512, (sub + 1) * 512)
                                qss = slice(qsl.start + sub * 512, qsl.start + (sub + 1) * 512)
                                nc.tensor.matmul(
                                    pscore[:, ss], k_sb[hsl, jsl], q_sb[hsl, qss],
                                    start=True, stop=True,
                                )
                            et = exp_pool.tile([P, FREE], bf16)
                            nc.scalar.activation(
                                et[:], pscore[:],
                                mybir.ActivationFunctionType.Exp, scale=0.125,
                            )
                            for sub in range(FREE // 512):
                                ss = slice(sub * 512, (sub + 1) * 512)
                                nc.tensor.matmul(
                                    pav[0 : DK + 1, ss],
                                    v_sb[:, b * SJT + jt, h * (DK + 1) : (h + 1) * (DK + 1)],
                                    et[:, ss],
                                    start=(jt == 0), stop=(jt == SJT - 1),
                                )
                        rc = rc_pool.tile([1, FREE], f32)
                        nc.vector.reciprocal(rc[:], pav[DK : DK + 1, :])
                        bc = bc_pool.tile([DK, FREE], f32)
                        nc.gpsimd.partition_broadcast(bc[:], rc[:])
                        nc.vector.tensor_mul(attn_sb[hsl, qsl], pav[0:DK, :], bc[:])

            # ---- phase 3: output projection (row-sharded Wo -> partial) ----
            for it in range(NIT):
                isl = slice(it * P, (it + 1) * P)
                po = pp_big.tile([P, FREE], f32, tag="big")
                for sub in range(FREE // 512):
                    ss = slice(sub * 512, (sub + 1) * 512)
                    nc.tensor.matmul(po[:, ss], attn_sb[:, isl], wo_sb[:, ss],
                                     start=True, stop=True)
                ot = out_pool.tile([P, D_MODEL], f32)
                nc.vector.tensor_copy(ot[:], po[:])
                nc.sync.dma_start(out[isl, :], ot[:])


def _build(loop=1):
    nc = bacc.Bacc("TRN2", target_bir_lowering=False, debug=False,
                   num_devices=N_CORES)
    xq = nc.dram_tensor("xq_t", [D_MODEL, BS], bf16, kind="ExternalInput").ap()
    xk = nc.dram_tensor("xk_t", [D_MODEL, BS], bf16, kind="ExternalInput").ap()
    xv = nc.dram_tensor("xv_t", [D_MODEL, BS], bf16, kind="ExternalInput").ap()
    wq = nc.dram_tensor("wq_t", [D_MODEL, DPC], bf16, kind="ExternalInput").ap()
    wk = nc.dram_tensor("wk_t", [D_MODEL, DPC], bf16, kind="ExternalInput").ap()
    wv = nc.dram_tensor("wv_t", [D_MODEL, DPC], bf16, kind="ExternalInput").ap()
    wo = nc.dram_tensor("wo_t", [DPC, D_MODEL], bf16, kind="ExternalInput").ap()
    bq = nc.dram_tensor("bq", [DPC, 1], f32, kind="ExternalInput").ap()
    bk = nc.dram_tensor("bk", [DPC, 1], f32, kind="ExternalInput").ap()
    out = nc.dram_tensor("out_p", [BS, D_MODEL], f32, kind="ExternalOutput").ap()

    with tile.TileContext(nc) as tc:
        _emit(tc, (xq, xk, xv, wq, wk, wv, wo, bq, bk, out), loop=loop)
    nc.compile()
    return nc


_cache = {}


def _get_nc(loop=1):
    if loop not in _cache:
        _cache[loop] = _build(loop)
    return _cache[loop]


def _make_in_maps(Q, K, V, Wq, bq, Wk, bk, Wv, bv, Wo, bo):
    xq_t = np.ascontiguousarray(np.asarray(Q, np.float32).reshape(BS, D_MODEL).T).astype(BF16)
    xk_t = np.ascontiguousarray(np.asarray(K, np.float32).reshape(BS, D_MODEL).T).astype(BF16)
    xv_t = np.ascontiguousarray(np.asarray(V, np.float32).reshape(BS, D_MODEL).T).astype(BF16)
    in_maps = []
    for c in range(N_CORES):
        sl = slice(c * DPC, (c + 1) * DPC)
        in_maps.append({
            "xq_t": xq_t, "xk_t": xk_t, "xv_t": xv_t,
            "wq_t": np.ascontiguousarray(np.asarray(Wq)[sl].T).astype(BF16),
            "wk_t": np.ascontiguousarray(np.asarray(Wk)[sl].T).astype(BF16),
            "wv_t": np.ascontiguousarray(np.asarray(Wv)[sl].T).astype(BF16),
            "wo_t": np.ascontiguousarray(np.asarray(Wo)[:, sl].T).astype(BF16),
            "bq": np.asarray(bq, np.float32)[sl].reshape(DPC, 1).copy(),
            "bk": np.asarray(bk, np.float32)[sl].reshape(DPC, 1).copy(),
        })
    return in_maps


def kernel(Q, K, V, Wq, bq, Wk, bk, Wv, bv, Wo, bo):
    nc = _get_nc()
    in_maps = _make_in_maps(Q, K, V, Wq, bq, Wk, bk, Wv, bv, Wo, bo)
    res = bass_utils.run_bass_kernel_spmd(nc, in_maps, core_ids=list(range(N_CORES)))
    acc = np.zeros((BS, D_MODEL), np.float32)
    for c in range(N_CORES):
        acc += res.results[c]["out_p"]
    corr = (np.asarray(bv, np.float64) @ np.asarray(Wo, np.float64).T
            + np.asarray(bo, np.float64)).astype(np.float32)
    return (acc + corr[None, :]).reshape(B, S, D_MODEL).astype(np.float32)
